# revision 18
# baseline (speedup 1.0000x reference)
"""AtomAttentionEncoder on 8 TRN2 NeuronCores (Bass/Tile).

Sharding: sequence-parallel over atoms/windows (core i owns atoms
[1024*i, 1024*(i+1)), windows [32*i, 32*(i+1)), output tokens
[256*i, 256*(i+1))).  Per layer, k/v projections are computed locally
(atom-major) and all-gathered so each core can row-gather its windows'
keys; the final atom->token mean pooling is a local token-sorted
banded one-hot matmul followed by a ReduceScatter over the token axis.

Self-contained: hardcodes all shapes from the problem spec.
"""

import math

import numpy as np

N, D, W, H, HEADS, L, T, FT = 8192, 128, 32, 128, 4, 3, 2048, 768
NW = N // W
DH = D // HEADS
NC = 8
NL = N // NC
WL = NW // NC
TL = T // NC
P = 128
NCH_F = NL // P
TB = T // P
INV_SQRT_DH = 1.0 / math.sqrt(DH)

_CACHE = {}


def _bf16():
    import ml_dtypes

    return ml_dtypes.bfloat16


def _build(nch_per_tb):
    import concourse.bacc as bacc
    import concourse.bass as bass
    import concourse.tile as tile
    from concourse import mybir
    from concourse.masks import make_identity

    f32 = mybir.dt.float32
    bf16 = mybir.dt.bfloat16
    i32 = mybir.dt.int32
    SIG = mybir.ActivationFunctionType.Sigmoid
    IDENT_F = mybir.ActivationFunctionType.Identity
    EXP = mybir.ActivationFunctionType.Exp
    SILU = mybir.ActivationFunctionType.Silu
    RELU = mybir.ActivationFunctionType.Relu
    SQUARE = mybir.ActivationFunctionType.Square
    SQRT = mybir.ActivationFunctionType.Sqrt

    nch_total = int(sum(nch_per_tb))
    nc = bacc.Bacc("TRN2", target_bir_lowering=False, debug=False, num_devices=NC)

    def din(name, shape, dt):
        return nc.dram_tensor(name, list(shape), dt, kind="ExternalInput").ap()

    qT_d = din("qT", (P, NL), f32)
    cT_d = din("cT", (P, NL), f32)
    rT_d = din("rT", (3, NL), bf16)
    rq_d = din("rq", (3, D), bf16)
    bias_d = din("bias", (P, WL * H), bf16)
    kidx_d = din("kidx", (H, WL), i32)
    sidx_d = din("sidx", (P, nch_total), i32)
    stid_d = din("stid", (P, nch_total), f32)
    invc_d = din("invc", (P, TL // P), f32)
    ws1_d = din("ws1", (D, L * D), bf16)
    wh1_d = din("wh1", (D, L * D), bf16)
    wq_d = din("wq", (D, L * D), bf16)
    bq_d = din("bq", (P, L), f32)
    wkv_d = din("wkv", (D, L * 2 * D), bf16)       # [wk | wv] per layer
    wg_d = din("wg", (D, L * D), bf16)
    wo_d = din("wo", (D, L * D), bf16)
    wop_d = din("wop", (D, L * D), bf16)
    bop_d = din("bop", (P, L), f32)
    ws2_d = din("ws2", (D, L * D), bf16)
    wh2_d = din("wh2", (D, L * D), bf16)
    wgate_d = din("wgate", (D, L * 2 * D), bf16)
    wlin_d = din("wlin", (D, L * 2 * D), bf16)
    w2a_d = din("w2a", (D, L * D), bf16)
    w2b_d = din("w2b", (D, L * D), bf16)
    wg2_d = din("wg2", (D, L * D), bf16)
    wa2t_d = din("wa2t", (D, FT), bf16)
    out_d = nc.dram_tensor("out", [TL, FT], f32, kind="ExternalOutput").ap()

    rg = [list(range(NC))]
    CH = (slice(0, 512), slice(512, 1024))

    with tile.TileContext(nc) as tc:
        with (
            tc.tile_pool(name="big", bufs=1) as big,
            tc.tile_pool(name="wpool", bufs=1) as wpool,
            tc.tile_pool(name="sb", bufs=2) as sb,
            tc.tile_pool(name="att", bufs=4) as att,
            tc.tile_pool(name="dram", bufs=1, space="DRAM") as dram,
        ):
            # warmup collective, sized/typed like the real kv AllGather
            warm_src = dram.tile([NL, 2 * D], bf16)
            warm_dst = dram.tile([N, 2 * D], bf16, addr_space="Shared")
            nc.gpsimd.collective_compute(
                "AllGather", mybir.AluOpType.bypass,
                ins=[warm_src[:].opt()], outs=[warm_dst[:].opt()],
                replica_groups=rg,
            )
            warm_dst2 = dram.tile([N, 2 * D], bf16, addr_space="Shared")
            nc.gpsimd.collective_compute(
                "AllGather", mybir.AluOpType.bypass,
                ins=[warm_src[:].opt()], outs=[warm_dst2[:].opt()],
                replica_groups=rg,
            )

            def load(pool, src_ap, shape, dt, name):
                t = pool.tile(list(shape), dt, name=name)
                nc.sync.dma_start(out=t[:], in_=src_ap)
                return t

            qT = load(big, qT_d[:], (P, NL), f32, "qT")
            cT = load(big, cT_d[:], (P, NL), f32, "cT")
            rT = load(big, rT_d[:], (3, NL), bf16, "rT")
            rq = load(big, rq_d[:], (3, D), bf16, "rq")
            bias_s = load(big, bias_d[:], (P, WL * H), bf16, "bias_s")
            kidx_s = load(big, kidx_d[:], (H, WL), i32, "kidx_s")
            sidx_s = load(big, sidx_d[:], (P, nch_total), i32, "sidx_s")
            stid_s = load(big, stid_d[:], (P, nch_total), f32, "stid_s")
            invc_s = load(big, invc_d[:], (P, TL // P), f32, "invc_s")

            wnames = dict(
                ws1=ws1_d, wh1=wh1_d, wq=wq_d, wg=wg_d, wo=wo_d, wop=wop_d,
                ws2=ws2_d, wh2=wh2_d, w2a=w2a_d, w2b=w2b_d, wg2=wg2_d,
            )
            wt = {k: load(wpool, v[:], (D, L * D), bf16, k) for k, v in wnames.items()}
            wkv = load(wpool, wkv_d[:], (D, L * 2 * D), bf16, "wkv")
            wgate = load(wpool, wgate_d[:], (D, L * 2 * D), bf16, "wgate")
            wlin = load(wpool, wlin_d[:], (D, L * 2 * D), bf16, "wlin")
            wa2t = load(wpool, wa2t_d[:], (D, FT), bf16, "wa2t")
            bq_s = load(wpool, bq_d[:], (P, L), f32, "bq_s")
            bop_s = load(wpool, bop_d[:], (P, L), f32, "bop_s")

            ident = big.tile([P, P], bf16, name="ident")
            make_identity(nc, ident[:])
            ones_col = big.tile([P, 1], bf16, name="ones_col")
            nc.vector.memset(ones_col[:], 1.0)
            one_row = big.tile([1, P], bf16, name="one_row")
            nc.vector.memset(one_row[:], 1.0)
            i128_row = big.tile([1, P], bf16, name="i128_row")
            nc.vector.memset(i128_row[:], 1.0 / 128.0)
            iota_i = big.tile([P, P], i32, name="iota_i")
            nc.gpsimd.iota(iota_i[:], pattern=[[1, P]], base=0, channel_multiplier=0)
            iota_f = big.tile([P, P], f32, name="iota_f")
            nc.vector.tensor_copy(iota_f[:], iota_i[:])
            eps_row = big.tile([1, 1], f32, name="eps_row")
            nc.vector.memset(eps_row[:], 1e-5)
            qbd_all = big.tile([P, WL * P], bf16, name="qbd_all")
            nc.vector.memset(qbd_all[:], 0.0)

            def ln_feature(x, name, pool):
                """LN over channels (partitions) of feature-major f32 x."""
                x_bf = sb.tile([P, NL], bf16, name=f"{name}_xbf", tag="lnxbf", bufs=1)
                nc.gpsimd.tensor_copy(x_bf[:], x[:])
                xsq = sb.tile([P, NL], bf16, name=f"{name}_sq", tag="lnsq", bufs=1)
                nc.scalar.square(xsq[:], x[:])
                psA = pool.tile([1, NL], f32, name=f"{name}_A", tag="pstatA", bufs=1)
                psB = pool.tile([1, NL], f32, name=f"{name}_B", tag="pstatB", bufs=1)
                for ci, sl in enumerate(CH):
                    nc.tensor.matmul(out=psA[:, sl], lhsT=ones_col[:],
                                     rhs=x_bf[:, sl], start=True, stop=True)
                    nc.tensor.matmul(out=psB[:, sl], lhsT=ones_col[:],
                                     rhs=xsq[:, sl], start=True, stop=True)
                m2 = sb.tile([1, NL], f32, name=f"{name}_m2", tag="lnm2", bufs=1)
                nc.scalar.activation(m2[:], psA[:], SQUARE,
                                     scale=float(1.0 / math.sqrt(128.0)))
                u = sb.tile([1, NL], f32, name=f"{name}_u", tag="lnu", bufs=1)
                nc.vector.tensor_tensor(out=u[:], in0=psB[:], in1=m2[:],
                                        op=mybir.AluOpType.subtract)
                sd = sb.tile([1, NL], f32, name=f"{name}_sd", tag="lnsd", bufs=1)
                nc.scalar.activation(sd[:], u[:], SQRT, bias=eps_row[:],
                                     scale=float(1.0 / 128.0))
                rstd = sb.tile([1, NL], f32, name=f"{name}_rs", tag="lnrs", bufs=1)
                nc.vector.reciprocal(rstd[:], sd[:])
                rstd_bf = sb.tile([1, NL], bf16, name=f"{name}_rb", tag="lnrb",
                                  bufs=1)
                nc.vector.tensor_copy(rstd_bf[:], rstd[:])
                mr = sb.tile([1, NL], bf16, name=f"{name}_mr", tag="lnmr", bufs=1)
                nc.vector.tensor_tensor(out=mr[:], in0=psA[:], in1=rstd[:],
                                        op=mybir.AluOpType.mult)
                out = sb.tile([P, NL], bf16, name=f"{name}_ln", tag="lnout")
                for ci, sl in enumerate(CH):
                    pbr = pool.tile([P, 512], f32, name=f"{name}_br{ci}",
                                    tag="pbc", bufs=2)
                    pbmr = pool.tile([P, 512], f32, name=f"{name}_bm{ci}",
                                     tag="pbc", bufs=2)
                    nc.tensor.matmul(out=pbr[:], lhsT=one_row[:],
                                     rhs=rstd_bf[:, sl], start=True, stop=True)
                    nc.tensor.matmul(out=pbmr[:], lhsT=i128_row[:], rhs=mr[:, sl],
                                     start=True, stop=True)
                    t = sb.tile([P, 512], f32, name=f"{name}_t{ci}", tag="lnt")
                    nc.vector.tensor_tensor(out=t[:], in0=x[:, sl], in1=pbr[:],
                                            op=mybir.AluOpType.mult)
                    nc.vector.tensor_tensor(out=out[:, sl], in0=t[:], in1=pbmr[:],
                                            op=mybir.AluOpType.subtract)
                return out

            def mm_feat(pool, lhsT_ap, rhs_tile, tag, bufs=2):
                outs = []
                for ci, sl in enumerate(CH):
                    pt = pool.tile([P, 512], f32, name=f"{tag}{ci}", tag="p512",
                                   bufs=bufs)
                    nc.tensor.matmul(out=pt[:], lhsT=lhsT_ap, rhs=rhs_tile[:, sl],
                                     start=True, stop=True)
                    outs.append(pt)
                return outs

            # ================= prelude =================
            with tc.tile_pool(name="psP", bufs=1, space="PSUM") as psP:
                a = big.tile([P, NL], f32, name="a")
                pa = mm_feat(psP, rq[:], rT, "pa")
                for ci, sl in enumerate(CH):
                    nc.vector.tensor_tensor(out=a[:, sl], in0=qT[:, sl],
                                            in1=pa[ci][:], op=mybir.AluOpType.add)
                c_n = ln_feature(cT, "cn", psP)
                c_bf = big.tile([P, NL], bf16, name="c_bf")
                nc.scalar.copy(c_bf[:], cT[:])

                def gate_from(rhs_tile, w_key, l, act, name, bias_col=None):
                    pg = mm_feat(psP, wt[w_key][:, D * l:D * (l + 1)], rhs_tile,
                                 f"pg_{name}")
                    out = big.tile([P, NL], bf16, name=name)
                    for ci in range(2):
                        kw = {}
                        if bias_col is not None:
                            kw["bias"] = bias_col
                        nc.scalar.activation(out[:, CH[ci]], pg[ci][:], act, **kw)
                    return out

                sg1 = [gate_from(c_n, "ws1", l, SIG, f"sg1_{l}") for l in range(L)]
                sh1 = [gate_from(c_n, "wh1", l, IDENT_F, f"sh1_{l}")
                       for l in range(L)]
                sg2 = [gate_from(c_n, "ws2", l, SIG, f"sg2_{l}") for l in range(L)]
                sh2 = [gate_from(c_n, "wh2", l, IDENT_F, f"sh2_{l}")
                       for l in range(L)]
                sop = [gate_from(c_bf, "wop", l, SIG, f"sop_{l}",
                                 bias_col=bop_s[:, l:l + 1]) for l in range(L)]
                sgc = [gate_from(c_bf, "wg2", l, SIG, f"sgc_{l}") for l in range(L)]

            # ================= layers =================
            for l in range(L):
                with tc.tile_pool(name=f"psL1_{l}", bufs=1, space="PSUM") as psL:
                    ln_a = ln_feature(a, f"l{l}a1", psL)
                    b_bf = sb.tile([P, NL], bf16, name=f"b_{l}", tag="b_bf",
                                   bufs=1)
                    bt = sb.tile([P, NL], bf16, name=f"bt_{l}", tag="bt", bufs=1)
                    nc.vector.tensor_tensor(out=bt[:], in0=sg1[l][:], in1=ln_a[:],
                                            op=mybir.AluOpType.mult)
                    nc.vector.tensor_tensor(out=b_bf[:], in0=bt[:], in1=sh1[l][:],
                                            op=mybir.AluOpType.add)

                    pq = mm_feat(psL, wt["wq"][:, D * l:D * (l + 1)], b_bf, "pq")
                    qh = sb.tile([P, NL], bf16, name=f"qh_{l}", tag="qh", bufs=1)
                    for ci in range(2):
                        nc.scalar.activation(qh[:, CH[ci]], pq[ci][:], IDENT_F,
                                             bias=bq_s[:, l:l + 1])
                    pgg = mm_feat(psL, wt["wg"][:, D * l:D * (l + 1)], b_bf, "pgg")
                    gs = sb.tile([P, NL], bf16, name=f"gs_{l}", tag="gs", bufs=1)
                    for ci in range(2):
                        nc.scalar.activation(gs[:, CH[ci]], pgg[ci][:], SIG)
                    qbd3 = qbd_all[:].rearrange("p (w m) -> p w m", w=WL)
                    qh3 = qh[:].rearrange("p (w q) -> p w q", w=WL)
                    for h in range(HEADS):
                        hs = slice(DH * h, DH * (h + 1))
                        nc.vector.tensor_copy(qbd3[hs, :, W * h:W * (h + 1)],
                                              qh3[hs, :, :])

                go = sb.tile([P, NL], bf16, name=f"go_{l}", tag="go", bufs=1)
                with tc.tile_pool(name=f"psA_{l}", bufs=1, space="PSUM") as psA:
                    kvam = sb.tile([P, NL * 2], bf16, name=f"kvam_{l}", tag="kvam",
                                   bufs=1)
                    for c in range(NCH_F):
                        pkv = psA.tile([P, 2 * D], f32, name=f"pkv_{l}_{c}",
                                       tag="pkvp", bufs=2)
                        nc.tensor.matmul(
                            out=pkv[:], lhsT=b_bf[:, P * c:P * (c + 1)],
                            rhs=wkv[:, 2 * D * l:2 * D * (l + 1)],
                            start=True, stop=True,
                        )
                        if c % 2 == 0:
                            nc.scalar.copy(kvam[:, 2 * D * c:2 * D * (c + 1)],
                                           pkv[:])
                        else:
                            nc.vector.tensor_copy(
                                kvam[:, 2 * D * c:2 * D * (c + 1)], pkv[:])
                    kv_bounce = dram.tile([NL, 2 * D], bf16, name=f"kvb_{l}")
                    nc.scalar.dma_start(
                        out=kv_bounce[:].rearrange("(c p) d -> p c d", p=P),
                        in_=kvam[:].rearrange("p (c d) -> p c d", c=NCH_F),
                    )
                    kv_full = dram.tile([N, 2 * D], bf16, addr_space="Shared",
                                        name=f"kv_full_{l}")
                    nc.gpsimd.collective_compute(
                        "AllGather", mybir.AluOpType.bypass,
                        ins=[kv_bounce[:].opt()], outs=[kv_full[:].opt()],
                        replica_groups=rg,
                    )
                    kv_t, khT_t, attn_t = {}, {}, {}
                    for it in range(WL + 3):
                        if it < WL:
                            w = it
                            kv = att.tile([P, 2 * D], bf16, name=f"kv_{l}_{w}",
                                          tag="kv", bufs=5)
                            nc.gpsimd.indirect_dma_start(
                                out=kv[:], out_offset=None, in_=kv_full[:],
                                in_offset=bass.IndirectOffsetOnAxis(
                                    ap=kidx_s[:, w:w + 1], axis=0),
                            )
                            kv_t[w] = kv
                        if it >= 3:
                            w = it - 3
                            attn = attn_t.pop(w)
                            pat = psA.tile([P, P], bf16, name=f"pat_{l}_{w}",
                                           tag="ptr", bufs=2)
                            nc.tensor.transpose(out=pat[:], in_=attn[:],
                                                identity=ident[:])
                            attnT = att.tile([P, P], bf16, name=f"atT_{l}_{w}",
                                             tag="attnT", bufs=2)
                            nc.vector.tensor_copy(attnT[:], pat[:])
                            po = psA.tile([P, W], f32, name=f"po_{l}_{w}",
                                          tag="po", bufs=2)
                            kvw = kv_t.pop(w)
                            for h in range(HEADS):
                                hs = slice(DH * h, DH * (h + 1))
                                nc.tensor.matmul(
                                    out=po[hs, :],
                                    lhsT=kvw[:, D + DH * h:D + DH * (h + 1)],
                                    rhs=attnT[:, hs], start=True, stop=True,
                                    tile_position=(0, DH * h),
                                )
                            nc.vector.tensor_tensor(
                                out=go[:, W * w:W * (w + 1)], in0=po[:],
                                in1=gs[:, W * w:W * (w + 1)],
                                op=mybir.AluOpType.mult,
                            )
                        if 2 <= it < WL + 2:
                            w = it - 2
                            plg = psA.tile([P, P], f32, name=f"plg_{l}_{w}",
                                           tag="plg", bufs=2)
                            nc.tensor.matmul(
                                out=plg[:], lhsT=ident[:],
                                rhs=bias_s[:, H * w:H * (w + 1)],
                                start=True, stop=False,
                            )
                            nc.tensor.matmul(
                                out=plg[:], lhsT=qbd_all[:, P * w:P * (w + 1)],
                                rhs=khT_t.pop(w)[:], start=False, stop=True,
                            )
                            attn = att.tile([P, H], bf16, name=f"at_{l}_{w}",
                                            tag="attn", bufs=3)
                            den = att.tile([P, 1], f32, name=f"den_{l}_{w}",
                                           tag="den", bufs=3)
                            nc.scalar.activation(attn[:], plg[:], EXP,
                                                 accum_out=den[:])
                            rden = att.tile([P, 1], f32, name=f"rden_{l}_{w}",
                                            tag="rden", bufs=3)
                            nc.vector.reciprocal(rden[:], den[:])
                            nc.vector.tensor_scalar(
                                out=attn[:], in0=attn[:], scalar1=rden[:, 0:1],
                                scalar2=None, op0=mybir.AluOpType.mult,
                            )
                            attn_t[w] = attn
                        if 1 <= it <= WL:
                            w = it - 1
                            ptr = psA.tile([P, P], bf16, name=f"ktr_{l}_{w}",
                                           tag="ptr", bufs=2)
                            nc.tensor.transpose(out=ptr[:], in_=kv_t[w][:, 0:D],
                                                identity=ident[:])
                            khT = att.tile([P, P], bf16, name=f"khT_{l}_{w}",
                                           tag="khT", bufs=3)
                            nc.scalar.copy(khT[:], ptr[:])
                            khT_t[w] = khT

                with tc.tile_pool(name=f"psL2_{l}", bufs=1, space="PSUM") as psT:
                    pop = mm_feat(psT, wt["wo"][:, D * l:D * (l + 1)], go, "pop")
                    for ci, sl in enumerate(CH):
                        tg = sb.tile([P, 512], f32, name=f"tg_{l}_{ci}", tag="tg")
                        nc.vector.tensor_tensor(out=tg[:], in0=pop[ci][:],
                                                in1=sop[l][:, sl],
                                                op=mybir.AluOpType.mult)
                        nc.vector.tensor_tensor(out=a[:, sl], in0=a[:, sl],
                                                in1=tg[:], op=mybir.AluOpType.add)

                    ln_a2 = ln_feature(a, f"l{l}a2", psT)
                    b2 = sb.tile([P, NL], bf16, name=f"b2_{l}", tag="b2", bufs=1)
                    bt2 = sb.tile([P, NL], bf16, name=f"bt2_{l}", tag="bt2",
                                  bufs=1)
                    nc.vector.tensor_tensor(out=bt2[:], in0=sg2[l][:],
                                            in1=ln_a2[:], op=mybir.AluOpType.mult)
                    nc.vector.tensor_tensor(out=b2[:], in0=bt2[:], in1=sh2[l][:],
                                            op=mybir.AluOpType.add)
                    for ci, sl in enumerate(CH):
                        hh = []
                        for hf in range(2):
                            base = 2 * D * l + D * hf
                            pgt = psT.tile([P, 512], f32,
                                           name=f"pgt_{l}_{ci}_{hf}", tag="p512",
                                           bufs=2)
                            nc.tensor.matmul(out=pgt[:],
                                             lhsT=wgate[:, base:base + D],
                                             rhs=b2[:, sl], start=True, stop=True)
                            plin = psT.tile([P, 512], f32,
                                            name=f"plin_{l}_{ci}_{hf}",
                                            tag="p512", bufs=2)
                            nc.tensor.matmul(out=plin[:],
                                             lhsT=wlin[:, base:base + D],
                                             rhs=b2[:, sl], start=True, stop=True)
                            hsil = sb.tile([P, 512], bf16,
                                           name=f"hsil_{l}_{ci}_{hf}", tag="hsil")
                            nc.scalar.activation(hsil[:], pgt[:], SILU)
                            hbf = sb.tile([P, 512], bf16,
                                          name=f"hbf_{l}_{ci}_{hf}", tag="hbf")
                            nc.vector.tensor_tensor(out=hbf[:], in0=hsil[:],
                                                    in1=plin[:],
                                                    op=mybir.AluOpType.mult)
                            hh.append(hbf)
                        pw2 = psT.tile([P, 512], f32, name=f"pw2_{l}_{ci}",
                                       tag="pbc", bufs=2)
                        nc.tensor.matmul(out=pw2[:],
                                         lhsT=wt["w2a"][:, D * l:D * (l + 1)],
                                         rhs=hh[0][:], start=True, stop=False)
                        nc.tensor.matmul(out=pw2[:],
                                         lhsT=wt["w2b"][:, D * l:D * (l + 1)],
                                         rhs=hh[1][:], start=False, stop=True)
                        tg2 = sb.tile([P, 512], f32, name=f"tg2_{l}_{ci}",
                                      tag="tg2")
                        nc.vector.tensor_tensor(out=tg2[:], in0=pw2[:],
                                                in1=sgc[l][:, sl],
                                                op=mybir.AluOpType.mult)
                        nc.vector.tensor_tensor(out=a[:, sl], in0=a[:, sl],
                                                in1=tg2[:],
                                                op=mybir.AluOpType.add)

            # ================= final: q2a + pooling =================
            a_bf = big.tile([P, NL], bf16, name="a_bf")
            nc.scalar.copy(a_bf[:], a[:])
            q2a_dram = dram.tile([NL, FT], bf16, name="q2a_dram")
            with tc.tile_pool(name="psF", bufs=2, space="PSUM") as psF:
                for c in range(NCH_F):
                    pq2 = psF.tile([P, FT], f32, name=f"pq2_{c}", tag="pq2")
                    nc.tensor.matmul(out=pq2[:, 0:512],
                                     lhsT=a_bf[:, P * c:P * (c + 1)],
                                     rhs=wa2t[:, 0:512], start=True, stop=True)
                    nc.tensor.matmul(out=pq2[:, 512:FT],
                                     lhsT=a_bf[:, P * c:P * (c + 1)],
                                     rhs=wa2t[:, 512:FT], start=True, stop=True)
                    q2s = sb.tile([P, FT], bf16, name=f"q2s_{c}", tag="q2s")
                    nc.scalar.activation(q2s[:], pq2[:], RELU)
                    nc.scalar.dma_start(out=q2a_dram[P * c:P * (c + 1), :],
                                        in_=q2s[:])

                partial = dram.tile([T, FT], bf16, name="partial")
                col = 0
                for tb in range(TB):
                    ppool = psF.tile([P, FT], f32, name=f"ppool_{tb}", tag="ppool")
                    nch = nch_per_tb[tb]
                    for j in range(nch):
                        q2g = sb.tile([P, FT], bf16, name=f"q2g_{tb}_{j}",
                                      tag="q2g", bufs=3)
                        nc.gpsimd.indirect_dma_start(
                            out=q2g[:], out_offset=None, in_=q2a_dram[:],
                            in_offset=bass.IndirectOffsetOnAxis(
                                ap=sidx_s[:, col:col + 1], axis=0),
                        )
                        oh = sb.tile([P, P], bf16, name=f"oh_{tb}_{j}", tag="oh",
                                     bufs=3)
                        nc.vector.tensor_scalar(
                            out=oh[:], in0=iota_f[:],
                            scalar1=stid_s[:, col:col + 1], scalar2=None,
                            op0=mybir.AluOpType.is_equal,
                        )
                        nc.tensor.matmul(out=ppool[:, 0:512], lhsT=oh[:],
                                         rhs=q2g[:, 0:512], start=(j == 0),
                                         stop=(j == nch - 1))
                        nc.tensor.matmul(out=ppool[:, 512:FT], lhsT=oh[:],
                                         rhs=q2g[:, 512:FT], start=(j == 0),
                                         stop=(j == nch - 1))
                        col += 1
                    pbf = sb.tile([P, FT], bf16, name=f"pbf_{tb}", tag="pbf")
                    if tb % 2 == 0:
                        nc.scalar.copy(pbf[:], ppool[:])
                    else:
                        nc.vector.tensor_copy(pbf[:], ppool[:])
                    nc.scalar.dma_start(out=partial[P * tb:P * (tb + 1), :],
                                        in_=pbf[:])

            rs_out = dram.tile([TL, FT], bf16, name="rs_out")
            nc.gpsimd.collective_compute(
                "ReduceScatter", mybir.AluOpType.add,
                ins=[partial[:].opt()], outs=[rs_out[:].opt()],
                replica_groups=rg,
            )
            for cc in range(TL // P):
                fin = sb.tile([P, FT], bf16, name=f"fin_{cc}", tag="fin")
                nc.scalar.dma_start(out=fin[:],
                                    in_=rs_out[P * cc:P * (cc + 1), :])
                fout = sb.tile([P, FT], f32, name=f"fout_{cc}", tag="fout")
                nc.vector.tensor_scalar(out=fout[:], in0=fin[:],
                                        scalar1=invc_s[:, cc:cc + 1],
                                        scalar2=None, op0=mybir.AluOpType.mult)
                nc.scalar.dma_start(out=out_d[P * cc:P * (cc + 1), :],
                                    in_=fout[:])

    nc.compile()
    return nc


def _prep_inputs(inputs):
    bf = _bf16()
    q = np.asarray(inputs["q"], np.float32)[0]
    c = np.asarray(inputs["c"], np.float32)[0]
    r = np.asarray(inputs["r"], np.float32)[0]
    bias = np.asarray(inputs["atom_enc_bias"], np.float32)[0]
    token_ids = np.asarray(inputs["token_ids"]).astype(np.int64).reshape(-1)
    key_indices = np.asarray(inputs["key_indices"]).astype(np.int64)[0]

    counts = np.bincount(token_ids, minlength=T).astype(np.float64)
    invc_full = (1.0 / (counts + 1e-6)).astype(np.float32)

    def w(name):
        return np.asarray(inputs[name], np.float32)

    def stackw(a3):
        return np.ascontiguousarray(a3.transpose(1, 0, 2).reshape(a3.shape[1], -1))

    wkv = np.concatenate([w("wk"), w("wv")], axis=2)   # (L, D, 2D)
    shared = {
        "rq": np.ascontiguousarray(w("r_to_q_w")).astype(bf),
        "ws1": stackw(w("adaln1_scale_w")).astype(bf),
        "wh1": stackw(w("adaln1_shift_w")).astype(bf),
        "wq": stackw(w("wq") * INV_SQRT_DH).astype(bf),
        "bq": np.ascontiguousarray((w("bq") * INV_SQRT_DH).T).astype(np.float32),
        "wkv": stackw(wkv).astype(bf),
        "wg": stackw(w("wg")).astype(bf),
        "wo": stackw(w("wo")).astype(bf),
        "wop": stackw(w("wop")).astype(bf),
        "bop": np.ascontiguousarray(w("bop").T).astype(np.float32),
        "ws2": stackw(w("adaln2_scale_w")).astype(bf),
        "wh2": stackw(w("adaln2_shift_w")).astype(bf),
        "wgate": stackw(w("w_gate")).astype(bf),
        "wlin": stackw(w("w_lin")).astype(bf),
        "w2a": stackw(w("w2")[:, 0:D, :]).astype(bf),
        "w2b": stackw(w("w2")[:, D:2 * D, :]).astype(bf),
        "wg2": stackw(w("wg2")).astype(bf),
        "wa2t": np.ascontiguousarray(w("w_a2t")).astype(bf),
    }

    per_core = []
    nch_per_tb = np.zeros(TB, np.int64)
    for i in range(NC):
        tloc = token_ids[NL * i:NL * (i + 1)]
        blocks = []
        for tb in range(TB):
            sel = np.nonzero((tloc >= P * tb) & (tloc < P * (tb + 1)))[0]
            blocks.append(sel)
            nch_per_tb[tb] = max(nch_per_tb[tb], (len(sel) + P - 1) // P)
        per_core.append(blocks)
    nch_per_tb = np.maximum(nch_per_tb, 1)
    nch_total = int(nch_per_tb.sum())

    in_maps = []
    for i in range(NC):
        asl = slice(NL * i, NL * (i + 1))
        tloc = token_ids[asl]
        sidx = np.zeros((P, nch_total), np.int32)
        stid = np.full((P, nch_total), -1.0, np.float32)
        col = 0
        for tb in range(TB):
            sel = per_core[i][tb]
            for j in range(int(nch_per_tb[tb])):
                part = sel[P * j:P * (j + 1)]
                sidx[0:len(part), col] = part.astype(np.int32)
                stid[0:len(part), col] = (tloc[part] - P * tb).astype(np.float32)
                col += 1
        m = {
            "qT": np.ascontiguousarray(q[asl].T),
            "cT": np.ascontiguousarray(c[asl].T),
            "rT": np.ascontiguousarray(r[asl].T).astype(bf),
            "bias": np.ascontiguousarray(
                bias[WL * i:WL * (i + 1)]
                .reshape(WL, HEADS * W, H).transpose(1, 0, 2).reshape(P, WL * H)
            ).astype(bf),
            "kidx": np.ascontiguousarray(
                key_indices[WL * i:WL * (i + 1)].T).astype(np.int32),
            "sidx": sidx,
            "stid": stid,
            "invc": np.ascontiguousarray(
                invc_full[TL * i:TL * (i + 1)].reshape(TL // P, P).T),
        }
        m.update(shared)
        in_maps.append(m)
    return in_maps, tuple(int(x) for x in nch_per_tb)


def _run(inputs, trace=False):
    from concourse.bass_utils import run_bass_kernel_spmd

    in_maps, nch_per_tb = _prep_inputs(inputs)
    if nch_per_tb not in _CACHE:
        _CACHE[nch_per_tb] = _build(nch_per_tb)
    nc = _CACHE[nch_per_tb]
    res = run_bass_kernel_spmd(nc, in_maps, core_ids=list(range(NC)), trace=trace)
    out = np.concatenate(
        [res.results[i]["out"] for i in range(NC)], axis=0).reshape(1, T, FT)
    return out, res


def kernel(**inputs):
    out, _ = _run(inputs, trace=False)
    return out


# revision 19
# speedup vs baseline: 1.2430x; 1.2430x over previous
"""AtomAttentionEncoder on 8 TRN2 NeuronCores (Bass/Tile).

Sharding: sequence-parallel over atoms/windows (core i owns atoms
[1024*i, 1024*(i+1)), windows [32*i, 32*(i+1)), output tokens
[256*i, 256*(i+1))).  Per layer, k/v projections are computed locally
(atom-major) and all-gathered so each core can row-gather its windows'
keys; the final atom->token mean pooling is a local token-sorted
banded one-hot matmul followed by a ReduceScatter over the token axis.

Self-contained: hardcodes all shapes from the problem spec.
"""

import math

import numpy as np

N, D, W, H, HEADS, L, T, FT = 8192, 128, 32, 128, 4, 3, 2048, 768
NW = N // W
DH = D // HEADS
NC = 8
NL = N // NC
WL = NW // NC
TL = T // NC
P = 128
NCH_F = NL // P
TB = T // P
INV_SQRT_DH = 1.0 / math.sqrt(DH)

_CACHE = {}


def _bf16():
    import ml_dtypes

    return ml_dtypes.bfloat16


def _build(nch_per_tb):
    import concourse.bacc as bacc
    import concourse.bass as bass
    import concourse.tile as tile
    from concourse import mybir
    from concourse.masks import make_identity

    f32 = mybir.dt.float32
    bf16 = mybir.dt.bfloat16
    i32 = mybir.dt.int32
    SIG = mybir.ActivationFunctionType.Sigmoid
    IDENT_F = mybir.ActivationFunctionType.Identity
    EXP = mybir.ActivationFunctionType.Exp
    SILU = mybir.ActivationFunctionType.Silu
    RELU = mybir.ActivationFunctionType.Relu
    SQUARE = mybir.ActivationFunctionType.Square
    SQRT = mybir.ActivationFunctionType.Sqrt

    nch_total = int(sum(nch_per_tb))
    nc = bacc.Bacc("TRN2", target_bir_lowering=False, debug=False, num_devices=NC)

    def din(name, shape, dt):
        return nc.dram_tensor(name, list(shape), dt, kind="ExternalInput").ap()

    qT_d = din("qT", (P, NL), f32)
    cT_d = din("cT", (P, NL), f32)
    rT_d = din("rT", (3, NL), bf16)
    rq_d = din("rq", (3, D), bf16)
    bias_d = din("bias", (P, WL * H), bf16)
    kidx_d = din("kidx", (H, WL), i32)
    sidx_d = din("sidx", (P, nch_total), i32)
    stid_d = din("stid", (P, nch_total), f32)
    invc_d = din("invc", (P, TL // P), f32)
    ws1_d = din("ws1", (D, L * D), bf16)
    wh1_d = din("wh1", (D, L * D), bf16)
    wq_d = din("wq", (D, L * D), bf16)
    bq_d = din("bq", (P, L), f32)
    wkv_d = din("wkv", (D, L * 2 * D), bf16)       # [wk | wv] per layer
    wg_d = din("wg", (D, L * D), bf16)
    wo_d = din("wo", (D, L * D), bf16)
    wop_d = din("wop", (D, L * D), bf16)
    bop_d = din("bop", (P, L), f32)
    ws2_d = din("ws2", (D, L * D), bf16)
    wh2_d = din("wh2", (D, L * D), bf16)
    wgate_d = din("wgate", (D, L * 2 * D), bf16)
    wlin_d = din("wlin", (D, L * 2 * D), bf16)
    w2a_d = din("w2a", (D, L * D), bf16)
    w2b_d = din("w2b", (D, L * D), bf16)
    wg2_d = din("wg2", (D, L * D), bf16)
    wa2t_d = din("wa2t", (D, FT), bf16)
    out_d = nc.dram_tensor("out", [TL, FT], f32, kind="ExternalOutput").ap()

    rg = [list(range(NC))]
    CH = (slice(0, 512), slice(512, 1024))

    with tile.TileContext(nc) as tc:
        with (
            tc.tile_pool(name="big", bufs=1) as big,
            tc.tile_pool(name="wpool", bufs=1) as wpool,
            tc.tile_pool(name="sb", bufs=2) as sb,
            tc.tile_pool(name="att", bufs=4) as att,
            tc.tile_pool(name="dram", bufs=1, space="DRAM") as dram,
        ):
            # warmup collective, sized/typed like the real kv AllGather
            warm_src = dram.tile([NL, 2 * D], bf16)
            warm_dst = dram.tile([N, 2 * D], bf16, addr_space="Shared")
            nc.gpsimd.collective_compute(
                "AllGather", mybir.AluOpType.bypass,
                ins=[warm_src[:].opt()], outs=[warm_dst[:].opt()],
                replica_groups=rg,
            )
            warm_dst2 = dram.tile([N, 2 * D], bf16, addr_space="Shared")
            nc.gpsimd.collective_compute(
                "AllGather", mybir.AluOpType.bypass,
                ins=[warm_src[:].opt()], outs=[warm_dst2[:].opt()],
                replica_groups=rg,
            )

            def load(pool, src_ap, shape, dt, name):
                t = pool.tile(list(shape), dt, name=name)
                nc.sync.dma_start(out=t[:], in_=src_ap)
                return t

            qT = load(big, qT_d[:], (P, NL), f32, "qT")
            cT = load(big, cT_d[:], (P, NL), f32, "cT")
            rT = load(big, rT_d[:], (3, NL), bf16, "rT")
            rq = load(big, rq_d[:], (3, D), bf16, "rq")
            bias_s = load(big, bias_d[:], (P, WL * H), bf16, "bias_s")
            kidx_s = load(big, kidx_d[:], (H, WL), i32, "kidx_s")
            sidx_s = load(big, sidx_d[:], (P, nch_total), i32, "sidx_s")
            stid_s = load(big, stid_d[:], (P, nch_total), f32, "stid_s")
            invc_s = load(big, invc_d[:], (P, TL // P), f32, "invc_s")

            wnames = dict(
                ws1=ws1_d, wh1=wh1_d, wq=wq_d, wg=wg_d, wo=wo_d, wop=wop_d,
                ws2=ws2_d, wh2=wh2_d, w2a=w2a_d, w2b=w2b_d, wg2=wg2_d,
            )
            wt = {k: load(wpool, v[:], (D, L * D), bf16, k) for k, v in wnames.items()}
            wkv = load(wpool, wkv_d[:], (D, L * 2 * D), bf16, "wkv")
            wgate = load(wpool, wgate_d[:], (D, L * 2 * D), bf16, "wgate")
            wlin = load(wpool, wlin_d[:], (D, L * 2 * D), bf16, "wlin")
            wa2t = load(wpool, wa2t_d[:], (D, FT), bf16, "wa2t")
            bq_s = load(wpool, bq_d[:], (P, L), f32, "bq_s")
            bop_s = load(wpool, bop_d[:], (P, L), f32, "bop_s")

            ident = big.tile([P, P], bf16, name="ident")
            make_identity(nc, ident[:])
            ones_col = big.tile([P, 1], bf16, name="ones_col")
            nc.vector.memset(ones_col[:], 1.0)
            one_row = big.tile([1, P], bf16, name="one_row")
            nc.vector.memset(one_row[:], 1.0)
            i128_row = big.tile([1, P], bf16, name="i128_row")
            nc.vector.memset(i128_row[:], 1.0 / 128.0)
            iota_i = big.tile([P, P], i32, name="iota_i")
            nc.gpsimd.iota(iota_i[:], pattern=[[1, P]], base=0, channel_multiplier=0)
            iota_f = big.tile([P, P], f32, name="iota_f")
            nc.vector.tensor_copy(iota_f[:], iota_i[:])
            eps_row = big.tile([1, 1], f32, name="eps_row")
            nc.vector.memset(eps_row[:], 1e-5)
            qbd_all = big.tile([P, WL * P], bf16, name="qbd_all")
            nc.vector.memset(qbd_all[:], 0.0)

            def ln_feature(x, name, pool):
                """LN over channels (partitions) of feature-major f32 x."""
                x_bf = sb.tile([P, NL], bf16, name=f"{name}_xbf", tag="lnxbf", bufs=1)
                nc.gpsimd.tensor_copy(x_bf[:], x[:])
                xsq = sb.tile([P, NL], bf16, name=f"{name}_sq", tag="lnsq", bufs=1)
                nc.scalar.square(xsq[:], x[:])
                psA = pool.tile([1, NL], f32, name=f"{name}_A", tag="pstatA", bufs=1)
                psB = pool.tile([1, NL], f32, name=f"{name}_B", tag="pstatB", bufs=1)
                for ci, sl in enumerate(CH):
                    nc.tensor.matmul(out=psA[:, sl], lhsT=ones_col[:],
                                     rhs=x_bf[:, sl], start=True, stop=True)
                    nc.tensor.matmul(out=psB[:, sl], lhsT=ones_col[:],
                                     rhs=xsq[:, sl], start=True, stop=True)
                m2 = sb.tile([1, NL], f32, name=f"{name}_m2", tag="lnm2", bufs=1)
                nc.scalar.activation(m2[:], psA[:], SQUARE,
                                     scale=float(1.0 / math.sqrt(128.0)))
                u = sb.tile([1, NL], f32, name=f"{name}_u", tag="lnu", bufs=1)
                nc.vector.tensor_tensor(out=u[:], in0=psB[:], in1=m2[:],
                                        op=mybir.AluOpType.subtract)
                sd = sb.tile([1, NL], f32, name=f"{name}_sd", tag="lnsd", bufs=1)
                nc.scalar.activation(sd[:], u[:], SQRT, bias=eps_row[:],
                                     scale=float(1.0 / 128.0))
                rstd = sb.tile([1, NL], f32, name=f"{name}_rs", tag="lnrs", bufs=1)
                nc.vector.reciprocal(rstd[:], sd[:])
                rstd_bf = sb.tile([1, NL], bf16, name=f"{name}_rb", tag="lnrb",
                                  bufs=1)
                nc.vector.tensor_copy(rstd_bf[:], rstd[:])
                mr = sb.tile([1, NL], bf16, name=f"{name}_mr", tag="lnmr", bufs=1)
                nc.vector.tensor_tensor(out=mr[:], in0=psA[:], in1=rstd[:],
                                        op=mybir.AluOpType.mult)
                out = sb.tile([P, NL], bf16, name=f"{name}_ln", tag="lnout")
                for ci, sl in enumerate(CH):
                    pbr = pool.tile([P, 512], f32, name=f"{name}_br{ci}",
                                    tag="pbc", bufs=2)
                    pbmr = pool.tile([P, 512], f32, name=f"{name}_bm{ci}",
                                     tag="pbc", bufs=2)
                    nc.tensor.matmul(out=pbr[:], lhsT=one_row[:],
                                     rhs=rstd_bf[:, sl], start=True, stop=True)
                    nc.tensor.matmul(out=pbmr[:], lhsT=i128_row[:], rhs=mr[:, sl],
                                     start=True, stop=True)
                    t = sb.tile([P, 512], f32, name=f"{name}_t{ci}", tag="lnt")
                    nc.vector.tensor_tensor(out=t[:], in0=x[:, sl], in1=pbr[:],
                                            op=mybir.AluOpType.mult)
                    nc.vector.tensor_tensor(out=out[:, sl], in0=t[:], in1=pbmr[:],
                                            op=mybir.AluOpType.subtract)
                return out

            def mm_feat(pool, lhsT_ap, rhs_tile, tag, bufs=2):
                outs = []
                for ci, sl in enumerate(CH):
                    pt = pool.tile([P, 512], f32, name=f"{tag}{ci}", tag="p512",
                                   bufs=bufs)
                    nc.tensor.matmul(out=pt[:], lhsT=lhsT_ap, rhs=rhs_tile[:, sl],
                                     start=True, stop=True)
                    outs.append(pt)
                return outs

            # ================= prelude =================
            with tc.tile_pool(name="psP", bufs=1, space="PSUM") as psP:
                a = big.tile([P, NL], f32, name="a")
                pa = mm_feat(psP, rq[:], rT, "pa")
                for ci, sl in enumerate(CH):
                    nc.vector.tensor_tensor(out=a[:, sl], in0=qT[:, sl],
                                            in1=pa[ci][:], op=mybir.AluOpType.add)
                c_n = ln_feature(cT, "cn", psP)
                c_bf = big.tile([P, NL], bf16, name="c_bf")
                nc.scalar.copy(c_bf[:], cT[:])

                def gate_from(rhs_tile, w_key, l, act, name, bias_col=None):
                    pg = mm_feat(psP, wt[w_key][:, D * l:D * (l + 1)], rhs_tile,
                                 f"pg_{name}")
                    out = big.tile([P, NL], bf16, name=name)
                    for ci in range(2):
                        kw = {}
                        if bias_col is not None:
                            kw["bias"] = bias_col
                        nc.scalar.activation(out[:, CH[ci]], pg[ci][:], act, **kw)
                    return out

                sg1 = [gate_from(c_n, "ws1", l, SIG, f"sg1_{l}") for l in range(L)]
                sh1 = [gate_from(c_n, "wh1", l, IDENT_F, f"sh1_{l}")
                       for l in range(L)]
                sg2 = [gate_from(c_n, "ws2", l, SIG, f"sg2_{l}") for l in range(L)]
                sh2 = [gate_from(c_n, "wh2", l, IDENT_F, f"sh2_{l}")
                       for l in range(L)]
                sop = [gate_from(c_bf, "wop", l, SIG, f"sop_{l}",
                                 bias_col=bop_s[:, l:l + 1]) for l in range(L)]
                sgc = [gate_from(c_bf, "wg2", l, SIG, f"sgc_{l}") for l in range(L)]

            # ================= layers =================
            for l in range(L):
                with tc.tile_pool(name=f"psL1_{l}", bufs=1, space="PSUM") as psL:
                    ln_a = ln_feature(a, f"l{l}a1", psL)
                    b_bf = sb.tile([P, NL], bf16, name=f"b_{l}", tag="b_bf",
                                   bufs=1)
                    bt = sb.tile([P, NL], bf16, name=f"bt_{l}", tag="bt", bufs=1)
                    nc.vector.tensor_tensor(out=bt[:], in0=sg1[l][:], in1=ln_a[:],
                                            op=mybir.AluOpType.mult)
                    nc.vector.tensor_tensor(out=b_bf[:], in0=bt[:], in1=sh1[l][:],
                                            op=mybir.AluOpType.add)

                    pq = mm_feat(psL, wt["wq"][:, D * l:D * (l + 1)], b_bf, "pq")
                    qh = sb.tile([P, NL], bf16, name=f"qh_{l}", tag="qh", bufs=1)
                    for ci in range(2):
                        nc.scalar.activation(qh[:, CH[ci]], pq[ci][:], IDENT_F,
                                             bias=bq_s[:, l:l + 1])
                    pgg = mm_feat(psL, wt["wg"][:, D * l:D * (l + 1)], b_bf, "pgg")
                    gs = sb.tile([P, NL], bf16, name=f"gs_{l}", tag="gs", bufs=1)
                    for ci in range(2):
                        nc.scalar.activation(gs[:, CH[ci]], pgg[ci][:], SIG)
                    qbd3 = qbd_all[:].rearrange("p (w m) -> p w m", w=WL)
                    qh3 = qh[:].rearrange("p (w q) -> p w q", w=WL)
                    for h in range(HEADS):
                        hs = slice(DH * h, DH * (h + 1))
                        nc.vector.tensor_copy(qbd3[hs, :, W * h:W * (h + 1)],
                                              qh3[hs, :, :])

                go = sb.tile([P, NL], bf16, name=f"go_{l}", tag="go", bufs=1)
                with tc.tile_pool(name=f"psA_{l}", bufs=1, space="PSUM") as psA:
                    kvam = sb.tile([P, NL * 2], bf16, name=f"kvam_{l}", tag="kvam",
                                   bufs=1)
                    for c in range(NCH_F):
                        pkv = psA.tile([P, 2 * D], f32, name=f"pkv_{l}_{c}",
                                       tag="pkvp", bufs=2)
                        nc.tensor.matmul(
                            out=pkv[:], lhsT=b_bf[:, P * c:P * (c + 1)],
                            rhs=wkv[:, 2 * D * l:2 * D * (l + 1)],
                            start=True, stop=True,
                        )
                        if c % 2 == 0:
                            nc.scalar.copy(kvam[:, 2 * D * c:2 * D * (c + 1)],
                                           pkv[:])
                        else:
                            nc.vector.tensor_copy(
                                kvam[:, 2 * D * c:2 * D * (c + 1)], pkv[:])
                    kv_bounce = dram.tile([NL, 2 * D], bf16, name=f"kvb_{l}")
                    nc.scalar.dma_start(
                        out=kv_bounce[:].rearrange("(c p) d -> p c d", p=P),
                        in_=kvam[:].rearrange("p (c d) -> p c d", c=NCH_F),
                    )
                    kv_full = dram.tile([N, 2 * D], bf16, addr_space="Shared",
                                        name=f"kv_full_{l}")
                    nc.gpsimd.collective_compute(
                        "AllGather", mybir.AluOpType.bypass,
                        ins=[kv_bounce[:].opt()], outs=[kv_full[:].opt()],
                        replica_groups=rg,
                    )
                    kv_t, khT_t, attn_t = {}, {}, {}
                    for it in range(WL + 3):
                        if it < WL:
                            w = it
                            kv = att.tile([P, 2 * D], bf16, name=f"kv_{l}_{w}",
                                          tag="kv", bufs=5)
                            nc.gpsimd.indirect_dma_start(
                                out=kv[:], out_offset=None, in_=kv_full[:],
                                in_offset=bass.IndirectOffsetOnAxis(
                                    ap=kidx_s[:, w:w + 1], axis=0),
                            )
                            kv_t[w] = kv
                        if it >= 3:
                            w = it - 3
                            attn = attn_t.pop(w)
                            pat = psA.tile([P, P], bf16, name=f"pat_{l}_{w}",
                                           tag="ptr", bufs=2)
                            nc.tensor.transpose(out=pat[:], in_=attn[:],
                                                identity=ident[:])
                            attnT = att.tile([P, P], bf16, name=f"atT_{l}_{w}",
                                             tag="attnT", bufs=2)
                            nc.vector.tensor_copy(attnT[:], pat[:])
                            po = psA.tile([P, W], f32, name=f"po_{l}_{w}",
                                          tag="po", bufs=2)
                            kvw = kv_t.pop(w)
                            for h in range(HEADS):
                                hs = slice(DH * h, DH * (h + 1))
                                nc.tensor.matmul(
                                    out=po[hs, :],
                                    lhsT=kvw[:, D + DH * h:D + DH * (h + 1)],
                                    rhs=attnT[:, hs], start=True, stop=True,
                                    tile_position=(0, DH * h),
                                )
                            nc.vector.tensor_tensor(
                                out=go[:, W * w:W * (w + 1)], in0=po[:],
                                in1=gs[:, W * w:W * (w + 1)],
                                op=mybir.AluOpType.mult,
                            )
                        if 2 <= it < WL + 2:
                            w = it - 2
                            plg = psA.tile([P, P], f32, name=f"plg_{l}_{w}",
                                           tag="plg", bufs=2)
                            nc.tensor.matmul(
                                out=plg[:], lhsT=qbd_all[:, P * w:P * (w + 1)],
                                rhs=khT_t.pop(w)[:], start=True, stop=True,
                            )
                            nc.vector.tensor_tensor(
                                out=plg[:], in0=plg[:],
                                in1=bias_s[:, H * w:H * (w + 1)],
                                op=mybir.AluOpType.add,
                            )
                            attn = att.tile([P, H], bf16, name=f"at_{l}_{w}",
                                            tag="attn", bufs=3)
                            den = att.tile([P, 1], f32, name=f"den_{l}_{w}",
                                           tag="den", bufs=3)
                            nc.scalar.activation(attn[:], plg[:], EXP,
                                                 accum_out=den[:])
                            rden = att.tile([P, 1], f32, name=f"rden_{l}_{w}",
                                            tag="rden", bufs=3)
                            nc.vector.reciprocal(rden[:], den[:])
                            nc.vector.tensor_scalar(
                                out=attn[:], in0=attn[:], scalar1=rden[:, 0:1],
                                scalar2=None, op0=mybir.AluOpType.mult,
                            )
                            attn_t[w] = attn
                        if 1 <= it <= WL:
                            w = it - 1
                            ptr = psA.tile([P, P], bf16, name=f"ktr_{l}_{w}",
                                           tag="ptr", bufs=2)
                            nc.tensor.transpose(out=ptr[:], in_=kv_t[w][:, 0:D],
                                                identity=ident[:])
                            khT = att.tile([P, P], bf16, name=f"khT_{l}_{w}",
                                           tag="khT", bufs=3)
                            nc.scalar.copy(khT[:], ptr[:])
                            khT_t[w] = khT

                with tc.tile_pool(name=f"psL2_{l}", bufs=1, space="PSUM") as psT:
                    pop = mm_feat(psT, wt["wo"][:, D * l:D * (l + 1)], go, "pop")
                    for ci, sl in enumerate(CH):
                        tg = sb.tile([P, 512], f32, name=f"tg_{l}_{ci}", tag="tg")
                        nc.vector.tensor_tensor(out=tg[:], in0=pop[ci][:],
                                                in1=sop[l][:, sl],
                                                op=mybir.AluOpType.mult)
                        nc.vector.tensor_tensor(out=a[:, sl], in0=a[:, sl],
                                                in1=tg[:], op=mybir.AluOpType.add)

                    ln_a2 = ln_feature(a, f"l{l}a2", psT)
                    b2 = sb.tile([P, NL], bf16, name=f"b2_{l}", tag="b2", bufs=1)
                    bt2 = sb.tile([P, NL], bf16, name=f"bt2_{l}", tag="bt2",
                                  bufs=1)
                    nc.vector.tensor_tensor(out=bt2[:], in0=sg2[l][:],
                                            in1=ln_a2[:], op=mybir.AluOpType.mult)
                    nc.vector.tensor_tensor(out=b2[:], in0=bt2[:], in1=sh2[l][:],
                                            op=mybir.AluOpType.add)
                    for ci, sl in enumerate(CH):
                        hh = []
                        for hf in range(2):
                            base = 2 * D * l + D * hf
                            pgt = psT.tile([P, 512], f32,
                                           name=f"pgt_{l}_{ci}_{hf}", tag="p512",
                                           bufs=2)
                            nc.tensor.matmul(out=pgt[:],
                                             lhsT=wgate[:, base:base + D],
                                             rhs=b2[:, sl], start=True, stop=True)
                            plin = psT.tile([P, 512], f32,
                                            name=f"plin_{l}_{ci}_{hf}",
                                            tag="p512", bufs=2)
                            nc.tensor.matmul(out=plin[:],
                                             lhsT=wlin[:, base:base + D],
                                             rhs=b2[:, sl], start=True, stop=True)
                            hsil = sb.tile([P, 512], bf16,
                                           name=f"hsil_{l}_{ci}_{hf}", tag="hsil")
                            nc.scalar.activation(hsil[:], pgt[:], SILU)
                            hbf = sb.tile([P, 512], bf16,
                                          name=f"hbf_{l}_{ci}_{hf}", tag="hbf")
                            nc.vector.tensor_tensor(out=hbf[:], in0=hsil[:],
                                                    in1=plin[:],
                                                    op=mybir.AluOpType.mult)
                            hh.append(hbf)
                        pw2 = psT.tile([P, 512], f32, name=f"pw2_{l}_{ci}",
                                       tag="pbc", bufs=2)
                        nc.tensor.matmul(out=pw2[:],
                                         lhsT=wt["w2a"][:, D * l:D * (l + 1)],
                                         rhs=hh[0][:], start=True, stop=False)
                        nc.tensor.matmul(out=pw2[:],
                                         lhsT=wt["w2b"][:, D * l:D * (l + 1)],
                                         rhs=hh[1][:], start=False, stop=True)
                        tg2 = sb.tile([P, 512], f32, name=f"tg2_{l}_{ci}",
                                      tag="tg2")
                        nc.vector.tensor_tensor(out=tg2[:], in0=pw2[:],
                                                in1=sgc[l][:, sl],
                                                op=mybir.AluOpType.mult)
                        nc.vector.tensor_tensor(out=a[:, sl], in0=a[:, sl],
                                                in1=tg2[:],
                                                op=mybir.AluOpType.add)

            # ================= final: q2a + pooling =================
            a_bf = big.tile([P, NL], bf16, name="a_bf")
            nc.scalar.copy(a_bf[:], a[:])
            q2a_dram = dram.tile([NL, FT], bf16, name="q2a_dram")
            with tc.tile_pool(name="psF", bufs=2, space="PSUM") as psF:
                for c in range(NCH_F):
                    pq2 = psF.tile([P, FT], f32, name=f"pq2_{c}", tag="pq2")
                    nc.tensor.matmul(out=pq2[:, 0:512],
                                     lhsT=a_bf[:, P * c:P * (c + 1)],
                                     rhs=wa2t[:, 0:512], start=True, stop=True)
                    nc.tensor.matmul(out=pq2[:, 512:FT],
                                     lhsT=a_bf[:, P * c:P * (c + 1)],
                                     rhs=wa2t[:, 512:FT], start=True, stop=True)
                    q2s = sb.tile([P, FT], bf16, name=f"q2s_{c}", tag="q2s")
                    nc.scalar.activation(q2s[:], pq2[:], RELU)
                    nc.scalar.dma_start(out=q2a_dram[P * c:P * (c + 1), :],
                                        in_=q2s[:])

                partial = dram.tile([T, FT], bf16, name="partial")
                col = 0
                for tb in range(TB):
                    ppool = psF.tile([P, FT], f32, name=f"ppool_{tb}", tag="ppool")
                    nch = nch_per_tb[tb]
                    for j in range(nch):
                        q2g = sb.tile([P, FT], bf16, name=f"q2g_{tb}_{j}",
                                      tag="q2g", bufs=3)
                        nc.gpsimd.indirect_dma_start(
                            out=q2g[:], out_offset=None, in_=q2a_dram[:],
                            in_offset=bass.IndirectOffsetOnAxis(
                                ap=sidx_s[:, col:col + 1], axis=0),
                        )
                        oh = sb.tile([P, P], bf16, name=f"oh_{tb}_{j}", tag="oh",
                                     bufs=3)
                        nc.vector.tensor_scalar(
                            out=oh[:], in0=iota_f[:],
                            scalar1=stid_s[:, col:col + 1], scalar2=None,
                            op0=mybir.AluOpType.is_equal,
                        )
                        nc.tensor.matmul(out=ppool[:, 0:512], lhsT=oh[:],
                                         rhs=q2g[:, 0:512], start=(j == 0),
                                         stop=(j == nch - 1))
                        nc.tensor.matmul(out=ppool[:, 512:FT], lhsT=oh[:],
                                         rhs=q2g[:, 512:FT], start=(j == 0),
                                         stop=(j == nch - 1))
                        col += 1
                    pbf = sb.tile([P, FT], bf16, name=f"pbf_{tb}", tag="pbf")
                    if tb % 2 == 0:
                        nc.scalar.copy(pbf[:], ppool[:])
                    else:
                        nc.vector.tensor_copy(pbf[:], ppool[:])
                    nc.scalar.dma_start(out=partial[P * tb:P * (tb + 1), :],
                                        in_=pbf[:])

            rs_out = dram.tile([TL, FT], bf16, name="rs_out")
            nc.gpsimd.collective_compute(
                "ReduceScatter", mybir.AluOpType.add,
                ins=[partial[:].opt()], outs=[rs_out[:].opt()],
                replica_groups=rg,
            )
            for cc in range(TL // P):
                fin = sb.tile([P, FT], bf16, name=f"fin_{cc}", tag="fin")
                nc.scalar.dma_start(out=fin[:],
                                    in_=rs_out[P * cc:P * (cc + 1), :])
                fout = sb.tile([P, FT], f32, name=f"fout_{cc}", tag="fout")
                nc.vector.tensor_scalar(out=fout[:], in0=fin[:],
                                        scalar1=invc_s[:, cc:cc + 1],
                                        scalar2=None, op0=mybir.AluOpType.mult)
                nc.scalar.dma_start(out=out_d[P * cc:P * (cc + 1), :],
                                    in_=fout[:])

    nc.compile()
    return nc


def _prep_inputs(inputs):
    bf = _bf16()
    q = np.asarray(inputs["q"], np.float32)[0]
    c = np.asarray(inputs["c"], np.float32)[0]
    r = np.asarray(inputs["r"], np.float32)[0]
    bias = np.asarray(inputs["atom_enc_bias"], np.float32)[0]
    token_ids = np.asarray(inputs["token_ids"]).astype(np.int64).reshape(-1)
    key_indices = np.asarray(inputs["key_indices"]).astype(np.int64)[0]

    counts = np.bincount(token_ids, minlength=T).astype(np.float64)
    invc_full = (1.0 / (counts + 1e-6)).astype(np.float32)

    def w(name):
        return np.asarray(inputs[name], np.float32)

    def stackw(a3):
        return np.ascontiguousarray(a3.transpose(1, 0, 2).reshape(a3.shape[1], -1))

    wkv = np.concatenate([w("wk"), w("wv")], axis=2)   # (L, D, 2D)
    shared = {
        "rq": np.ascontiguousarray(w("r_to_q_w")).astype(bf),
        "ws1": stackw(w("adaln1_scale_w")).astype(bf),
        "wh1": stackw(w("adaln1_shift_w")).astype(bf),
        "wq": stackw(w("wq") * INV_SQRT_DH).astype(bf),
        "bq": np.ascontiguousarray((w("bq") * INV_SQRT_DH).T).astype(np.float32),
        "wkv": stackw(wkv).astype(bf),
        "wg": stackw(w("wg")).astype(bf),
        "wo": stackw(w("wo")).astype(bf),
        "wop": stackw(w("wop")).astype(bf),
        "bop": np.ascontiguousarray(w("bop").T).astype(np.float32),
        "ws2": stackw(w("adaln2_scale_w")).astype(bf),
        "wh2": stackw(w("adaln2_shift_w")).astype(bf),
        "wgate": stackw(w("w_gate")).astype(bf),
        "wlin": stackw(w("w_lin")).astype(bf),
        "w2a": stackw(w("w2")[:, 0:D, :]).astype(bf),
        "w2b": stackw(w("w2")[:, D:2 * D, :]).astype(bf),
        "wg2": stackw(w("wg2")).astype(bf),
        "wa2t": np.ascontiguousarray(w("w_a2t")).astype(bf),
    }

    per_core = []
    nch_per_tb = np.zeros(TB, np.int64)
    for i in range(NC):
        tloc = token_ids[NL * i:NL * (i + 1)]
        blocks = []
        for tb in range(TB):
            sel = np.nonzero((tloc >= P * tb) & (tloc < P * (tb + 1)))[0]
            blocks.append(sel)
            nch_per_tb[tb] = max(nch_per_tb[tb], (len(sel) + P - 1) // P)
        per_core.append(blocks)
    nch_per_tb = np.maximum(nch_per_tb, 1)
    nch_total = int(nch_per_tb.sum())

    in_maps = []
    for i in range(NC):
        asl = slice(NL * i, NL * (i + 1))
        tloc = token_ids[asl]
        sidx = np.zeros((P, nch_total), np.int32)
        stid = np.full((P, nch_total), -1.0, np.float32)
        col = 0
        for tb in range(TB):
            sel = per_core[i][tb]
            for j in range(int(nch_per_tb[tb])):
                part = sel[P * j:P * (j + 1)]
                sidx[0:len(part), col] = part.astype(np.int32)
                stid[0:len(part), col] = (tloc[part] - P * tb).astype(np.float32)
                col += 1
        m = {
            "qT": np.ascontiguousarray(q[asl].T),
            "cT": np.ascontiguousarray(c[asl].T),
            "rT": np.ascontiguousarray(r[asl].T).astype(bf),
            "bias": np.ascontiguousarray(
                bias[WL * i:WL * (i + 1)]
                .reshape(WL, HEADS * W, H).transpose(1, 0, 2).reshape(P, WL * H)
            ).astype(bf),
            "kidx": np.ascontiguousarray(
                key_indices[WL * i:WL * (i + 1)].T).astype(np.int32),
            "sidx": sidx,
            "stid": stid,
            "invc": np.ascontiguousarray(
                invc_full[TL * i:TL * (i + 1)].reshape(TL // P, P).T),
        }
        m.update(shared)
        in_maps.append(m)
    return in_maps, tuple(int(x) for x in nch_per_tb)


def _run(inputs, trace=False):
    from concourse.bass_utils import run_bass_kernel_spmd

    in_maps, nch_per_tb = _prep_inputs(inputs)
    if nch_per_tb not in _CACHE:
        _CACHE[nch_per_tb] = _build(nch_per_tb)
    nc = _CACHE[nch_per_tb]
    res = run_bass_kernel_spmd(nc, in_maps, core_ids=list(range(NC)), trace=trace)
    out = np.concatenate(
        [res.results[i]["out"] for i in range(NC)], axis=0).reshape(1, T, FT)
    return out, res


def kernel(**inputs):
    out, _ = _run(inputs, trace=False)
    return out
